# revision 10
# baseline (speedup 1.0000x reference)
"""Trainium2 Bass kernel for EuclideanCodebook (VQ) forward + EMA update.

kernel(**inputs) takes FULL inputs
  x (8, 4096, 128) f32, embed (1, 8192, 128) f32,
  cluster_size (1, 8192) f32, embed_avg (1, 8192, 128) f32
returns the full tuple
  (quantized (32768, 1, 128), indices (32768,) i32, dist (32768, 8192) f32,
   new_embed (1, 8192, 128), new_cluster_size (1, 8192), new_embed_avg (1, 8192, 128))

Sharding: data-parallel over tokens (4096/core x 8 cores), codebook replicated,
per-core partial segment sums all-reduced on-device, EMA computed on-device.

Math per core:
  PSUM p = 2x.e - ||e||^2 via fp16 hi/lo split matmuls (fp32-grade accuracy)
  dist = x2 - p on ScalarE during PSUM->SBUF copy (per-partition bias)
  argmin: per-512-block mins on DVE -> winning 1024-block -> DMA-gather the
  block from the dist output in DRAM -> exact local min + position (max_index)
  segment sums: one-hot (GpSimd) x PE matmuls accumulating es^T / counts
"""
import sys
sys.path.insert(0, "/opt/trn_rl_repo")
import numpy as np

from concourse import bass, bacc, tile, mybir, bass_isa
from concourse.bass_utils import run_bass_kernel_spmd

F32 = mybir.dt.float32
F16 = mybir.dt.float16
I16 = mybir.dt.int16
I32 = mybir.dt.int32
U32 = mybir.dt.uint32
AF = mybir.ActivationFunctionType
OP = mybir.AluOpType
AX = mybir.AxisListType

NCORES = 8
T = 4096
K = 8192
D = 128
NT = T // 128      # 32 token tiles
NKB = K // 512     # 16 k-blocks
DECAY = 0.1
EPS = 1e-5

_CACHE = {}


def _build(num_devices=NCORES, use_collectives=True, do_rescue=True, do_phase2=True, do_distdma=True, do_blockmin=True):
    nc = bacc.Bacc("TRN2", target_bir_lowering=False, debug=False,
                   enable_asserts=False, num_devices=num_devices)
    x_d = nc.dram_tensor("x", [T, D], F32, kind="ExternalInput").ap()
    e_d = nc.dram_tensor("embed", [K, D], F32, kind="ExternalInput").ap()
    cs_d = nc.dram_tensor("cluster_size", [1, K], F32, kind="ExternalInput").ap()
    ea_d = nc.dram_tensor("embed_avg", [K, D], F32, kind="ExternalInput").ap()

    dist_d = nc.dram_tensor("dist", [T, K], F32, kind="ExternalOutput").ap()
    ind_d = nc.dram_tensor("indices", [NT, 128], I32, kind="ExternalOutput").ap()
    qt_d = nc.dram_tensor("quantized", [T, D], F32, kind="ExternalOutput").ap()
    nemb_d = nc.dram_tensor("new_embed", [K, D], F32, kind="ExternalOutput").ap()
    ncs_d = nc.dram_tensor("new_cluster_size", [1, K], F32, kind="ExternalOutput").ap()
    navg_d = nc.dram_tensor("new_embed_avg", [K, D], F32, kind="ExternalOutput").ap()

    with tile.TileContext(nc) as tc:
        with (
            tc.tile_pool(name="cst", bufs=1) as cst,
            tc.tile_pool(name="stage", bufs=2) as stage,
            tc.tile_pool(name="dtile", bufs=6) as dtile,
            tc.tile_pool(name="small", bufs=3) as small,
            tc.tile_pool(name="ps", bufs=1, space="PSUM") as ps,
            tc.tile_pool(name="dram", bufs=1, space="DRAM") as dp,
        ):
            # =============== constants ===============
            ones16 = cst.tile([128, 128], F16, tag="ones16")
            ident = cst.tile([128, 128], F16, tag="ident")
            nc.vector.memset(ones16[:], 1.0)
            nc.gpsimd.affine_select(ident[:], ones16[:], pattern=[[1, 128]],
                                    compare_op=OP.is_equal, fill=0.0,
                                    base=0, channel_multiplier=-1)
            identf = cst.tile([128, 128], F32, tag="identf")
            nc.vector.tensor_copy(identf[:], ident[:])
            ones2 = cst.tile([2, 128], F16, tag="ones2")
            nc.vector.memset(ones2[:], 1.0)
            onescol = cst.tile([128, 1], F16, tag="onescol")
            nc.vector.memset(onescol[:], 1.0)
            qcol = cst.tile([128, 1], F32, tag="qcol")
            nc.vector.memset(qcol[:], 0.25)

            # =============== x: load chunked, split fp16 hi/lo, transpose, x2 ===============
            xhi = cst.tile([128, NT, D], F16, tag="xhi")
            xlo = cst.tile([128, NT, D], F16, tag="xlo")
            xThi = cst.tile([128, T], F16, tag="xThi")
            xTlo = cst.tile([128, T], F16, tag="xTlo")
            x2c = cst.tile([128, NT], F32, tag="x2c")
            NXC = 4
            xr = x_d.rearrange("(j p) d -> p j d", p=128)
            for c in range(NT // NXC):
                x_st = stage.tile([128, NXC, D], F32, tag="x_st")
                nc.sync.dma_start(out=x_st[:], in_=xr[:, c * NXC:(c + 1) * NXC, :])
                jsl = slice(c * NXC, (c + 1) * NXC)
                nc.vector.tensor_copy(xhi[:, jsl, :], x_st[:])
                nc.vector.tensor_tensor(xlo[:, jsl, :], x_st[:], xhi[:, jsl, :],
                                        op=OP.subtract)
                for q in range(NXC):
                    j = c * NXC + q
                    scr = stage.tile([128, D], F32, tag="sq_scr")
                    nc.scalar.activation(scr[:], x_st[:, q, :], AF.Square,
                                         accum_out=x2c[:, j:j + 1])
                    pt = ps.tile([128, 128], F16, tag="pb4")
                    nc.tensor.transpose(pt[:], xhi[:, j, :], ident[:])
                    nc.scalar.activation(xThi[:, j * 128:(j + 1) * 128], pt[:], AF.Copy)
                    pt2 = ps.tile([128, 128], F16, tag="pb5")
                    nc.tensor.transpose(pt2[:], xlo[:, j, :], ident[:])
                    nc.scalar.activation(xTlo[:, j * 128:(j + 1) * 128], pt2[:], AF.Copy)

            # =============== E = 2*embed: load chunked, split, transpose ===============
            EThi = cst.tile([128, K], F16, tag="EThi")
            ETlo = cst.tile([128, K], F16, tag="ETlo")
            NEC = 4
            er = e_d.rearrange("(j p) d -> p j d", p=128)
            for c in range((K // 128) // NEC):
                e_st = stage.tile([128, NEC, D], F32, tag="e_st")
                nc.sync.dma_start(out=e_st[:], in_=er[:, c * NEC:(c + 1) * NEC, :])
                e2x = stage.tile([128, NEC, D], F32, tag="e2x")
                nc.vector.tensor_scalar_mul(e2x[:], e_st[:], 2.0)
                ehi = stage.tile([128, NEC, D], F16, tag="ehi")
                elo = stage.tile([128, NEC, D], F16, tag="elo")
                nc.vector.tensor_copy(ehi[:], e2x[:])
                nc.vector.tensor_tensor(elo[:], e2x[:], ehi[:], op=OP.subtract)
                for q in range(NEC):
                    j = c * NEC + q
                    pt = ps.tile([128, 128], F16, tag="pb4")
                    nc.tensor.transpose(pt[:], ehi[:, q, :], ident[:])
                    nc.scalar.activation(EThi[:, j * 128:(j + 1) * 128], pt[:], AF.Copy)
                    pt2 = ps.tile([128, 128], F16, tag="pb5")
                    nc.tensor.transpose(pt2[:], elo[:, q, :], ident[:])
                    nc.scalar.activation(ETlo[:, j * 128:(j + 1) * 128], pt2[:], AF.Copy)

            # =============== -||e||^2 as fp16 (hi, lo) rows ===============
            ne2pair = cst.tile([2, K], F16, tag="ne2pair")
            for b in range(NKB):
                sl = slice(b * 512, (b + 1) * 512)
                es_ = stage.tile([128, 512], F32, tag="es_")
                nc.vector.tensor_tensor(es_[:], EThi[:, sl], ETlo[:, sl], op=OP.add)
                esq = stage.tile([128, 512], F32, tag="esq")
                nc.vector.tensor_tensor(esq[:], es_[:], es_[:], op=OP.mult)
                pe2 = ps.tile([1, 512], F32, tag="pb6")
                nc.tensor.matmul(pe2[:], qcol[:], esq[:], start=True, stop=True)
                ne2f = stage.tile([1, 512], F32, tag="ne2f")
                nc.scalar.activation(ne2f[:], pe2[:], AF.Copy, scale=-1.0)
                nc.vector.tensor_copy(ne2pair[0:1, sl], ne2f[:])
                ne2lo = stage.tile([1, 512], F16, tag="ne2lo")
                nc.vector.tensor_tensor(ne2lo[:], ne2f[:], ne2pair[0:1, sl],
                                        op=OP.subtract)
                nc.sync.dma_start(out=ne2pair[1:2, sl], in_=ne2lo[:])

            # =============== dist loop ===============
            kf32 = cst.tile([128, NT], F32, tag="kf32")
            b10all = cst.tile([128, NT], F32, tag="b10all")
            bmin = cst.tile([128, NT, 8], F32, tag="bmin")
            idxg_dram = dp.tile([NT, 128], I16, tag="idxg_dram")

            for j in range(NT):
                tsl = slice(j * 128, (j + 1) * 128)
                bm512 = small.tile([128, 16], F32, tag="bm512")
                for b in range(NKB):
                    sl = slice(b * 512, (b + 1) * 512)
                    pm = ps.tile([128, 512], F32, tag="pb%d" % (b % 4))
                    nc.tensor.matmul(pm[:], xThi[:, tsl], EThi[:, sl], start=True, stop=False)
                    nc.tensor.matmul(pm[:], xThi[:, tsl], ETlo[:, sl], start=False, stop=False)
                    nc.tensor.matmul(pm[:], xTlo[:, tsl], EThi[:, sl], start=False, stop=False)
                    nc.tensor.matmul(pm[:], ones2[:], ne2pair[:, sl], start=False, stop=True)
                    dt = dtile.tile([128, 512], F32, tag="dt")
                    nc.scalar.activation(dt[:], pm[:], AF.Identity,
                                         bias=x2c[:, j:j + 1], scale=-1.0)
                    if do_distdma:
                        nc.sync.dma_start(out=dist_d[tsl, sl], in_=dt[:])
                    if do_blockmin:
                        nc.vector.tensor_reduce(bm512[:, b:b + 1], dt[:], axis=AX.X, op=OP.min)
                if not do_blockmin:
                    nc.vector.memset(bm512[:], 0.0)
                nc.vector.tensor_tensor(bmin[:, j, :], bm512[:, 0::2], bm512[:, 1::2],
                                        op=OP.min)
                gmin = small.tile([128, 1], F32, tag="gmin")
                nc.vector.tensor_reduce(gmin[:], bmin[:, j, :], axis=AX.X, op=OP.min)
                b10 = small.tile([128, 8], U32, tag="b10")
                nc.vector.max_index(b10[:], gmin[:].broadcast_to([128, 8]), bmin[:, j, :])
                nc.vector.tensor_copy(b10all[:, j:j + 1], b10[:, 0:1])
                # gather idx = t*8 + b10  (int16; max 32767)
                piota = small.tile([128, 1], F32, tag="piota")
                nc.gpsimd.iota(piota[:], pattern=[[1, 1]], base=j * 128,
                               channel_multiplier=1,
                               allow_small_or_imprecise_dtypes=True)
                gidxf = small.tile([128, 1], F32, tag="gidxf")
                nc.vector.tensor_scalar(gidxf[:], piota[:], 8.0, None, op0=OP.mult)
                nc.vector.tensor_tensor(gidxf[:], gidxf[:], b10all[:, j:j + 1], op=OP.add)
                gidx16 = small.tile([128, 1], I16, tag="gidx16")
                nc.vector.tensor_copy(gidx16[:], gidxf[:])
                nc.sync.dma_start(out=idxg_dram[j:j + 1, :].rearrange("a p -> p a"),
                                  in_=gidx16[:])

            # =============== rescue: exact position within winning 1024-block ===============
            dist_blocks = dist_d.rearrange("t (b q) -> (t b) q", q=1024)
            for j in (range(NT) if do_rescue else []):
                idxs = small.tile([128, 8], I16, tag="r_idxs")
                for rr in range(8):
                    nc.sync.dma_start(
                        out=idxs[16 * rr:16 * (rr + 1), :],
                        in_=idxg_dram[j:j + 1, :].rearrange("a (s p) -> p (a s)", p=16))
                gbuf = stage.tile([128, 1, 1024], F32, tag="gbuf")
                nc.gpsimd.dma_gather(gbuf[:], dist_blocks, idxs[:], num_idxs=128,
                                     num_idxs_reg=128, elem_size=1024)
                lmin = small.tile([128, 1], F32, tag="lmin")
                nc.vector.tensor_reduce(lmin[:], gbuf[:, 0, :], axis=AX.X, op=OP.min)
                lpos = small.tile([128, 8], U32, tag="lpos")
                nc.vector.max_index(lpos[:], lmin[:].broadcast_to([128, 8]), gbuf[:, 0, :])
                lposf = small.tile([128, 1], F32, tag="lposf")
                nc.vector.tensor_copy(lposf[:], lpos[:, 0:1])
                bb = small.tile([128, 1], F32, tag="bb")
                nc.vector.tensor_scalar(bb[:], b10all[:, j:j + 1], 1024.0, None,
                                        op0=OP.mult)
                nc.vector.tensor_tensor(kf32[:, j:j + 1], bb[:], lposf[:], op=OP.add)

            if not do_rescue:
                nc.vector.tensor_copy(kf32[:], b10all[:])
            # indices out (int32), token t = 128*j + p  ->  ind_d[j, p]
            ki32 = cst.tile([128, NT], I32, tag="ki32")
            nc.vector.tensor_copy(ki32[:], kf32[:])
            nc.sync.dma_start(out=ind_d.rearrange("j p -> p j"), in_=ki32[:])

            # k as wrapped int16 idx list for the embed gather
            k16 = small.tile([128, NT], I16, tag="k16")
            nc.vector.tensor_copy(k16[:], kf32[:])
            k16_dram = dp.tile([NT, 128], I16, tag="k16_dram")
            nc.sync.dma_start(out=k16_dram[:].rearrange("j p -> p j"), in_=k16[:])
            kidx = cst.tile([128, T // 16], I16, tag="kidx")
            for rr in range(8):
                nc.sync.dma_start(
                    out=kidx[16 * rr:16 * (rr + 1), :],
                    in_=k16_dram[:].rearrange("j (s p) -> p (j s)", p=16))

            # =============== quantized = embed[k] ===============
            NQC = 8   # 1024 idxs per gather
            for c in range(T // (128 * NQC)):
                qbuf = stage.tile([128, NQC, D], F32, tag="qbuf")
                nc.gpsimd.dma_gather(qbuf[:], e_d[:],
                                     kidx[:, c * (128 * NQC) // 16:(c + 1) * (128 * NQC) // 16],
                                     num_idxs=128 * NQC, num_idxs_reg=128 * NQC,
                                     elem_size=D)
                nc.sync.dma_start(
                    out=qt_d.rearrange("(j p) d -> p j d", p=128)[:, c * NQC:(c + 1) * NQC, :],
                    in_=qbuf[:])

            # =============== phase 2: segment sums (es^T, counts) ===============
            esT_dram = dp.tile([128, K], F32, tag="esT_dram")
            cs_dram = dp.tile([1, K], F32, tag="cs_dram")
            if not do_phase2:
                zz512 = stage.tile([128, 512], F32, tag="esd")
                nc.vector.memset(zz512[:], 0.0)
                for b in range(NKB):
                    nc.sync.dma_start(out=esT_dram[:, b * 512:(b + 1) * 512], in_=zz512[:])
                for b in range(0, NKB):
                    nc.sync.dma_start(out=cs_dram[:, b * 512:(b + 1) * 512], in_=zz512[0:1, :])
            for g in (range(4) if do_phase2 else []):
                iotg = stage.tile([128, 2048], F32, tag="iotg", bufs=1)
                nc.gpsimd.iota(iotg[:], pattern=[[1, 2048]], base=g * 2048,
                               channel_multiplier=0,
                               allow_small_or_imprecise_dtypes=True)
                esps = [ps.tile([128, 512], F32, tag="pb%d" % q, name="esps%d" % q) for q in range(4)]
                csps = [ps.tile([1, 512], F32, tag="pb%d" % (4 + q), name="csps%d" % q) for q in range(4)]
                for j in range(NT):
                    oh = stage.tile([128, 2048], F16, tag="oh")
                    nc.gpsimd.tensor_scalar(oh[:], iotg[:], kf32[:, j:j + 1], None,
                                            op0=OP.is_equal)
                    st = (j == 0)
                    sp = (j == NT - 1)
                    for q in range(4):
                        osl = slice(q * 512, (q + 1) * 512)
                        nc.tensor.matmul(esps[q][:], xhi[:, j, :], oh[:, osl],
                                         start=st, stop=False)
                        nc.tensor.matmul(esps[q][:], xlo[:, j, :], oh[:, osl],
                                         start=False, stop=sp)
                        nc.tensor.matmul(csps[q][:], onescol[:], oh[:, osl],
                                         start=st, stop=sp)
                for q in range(4):
                    b = g * 4 + q
                    sl = slice(b * 512, (b + 1) * 512)
                    esd = stage.tile([128, 512], F32, tag="esd")
                    nc.scalar.activation(esd[:], esps[q][:], AF.Copy)
                    nc.sync.dma_start(out=esT_dram[:, sl], in_=esd[:])
                    csd = stage.tile([1, 512], F32, tag="csd")
                    nc.scalar.activation(csd[:], csps[q][:], AF.Copy)
                    nc.sync.dma_start(out=cs_dram[:, sl], in_=csd[:])

            # =============== all-reduce ===============
            esT_red = dp.tile([128, K], F32, tag="esT_red")
            cs_red = dp.tile([1, K], F32, tag="cs_red")
            if use_collectives:
                nc.gpsimd.collective_compute("AllReduce", OP.add,
                                             replica_groups=[list(range(NCORES))],
                                             ins=[esT_dram[:].opt()], outs=[esT_red[:].opt()])
                nc.gpsimd.collective_compute("AllReduce", OP.add,
                                             replica_groups=[list(range(NCORES))],
                                             ins=[cs_dram[:].opt()], outs=[cs_red[:].opt()])
            else:
                nc.sync.dma_start(out=esT_red[:], in_=esT_dram[:])
                nc.sync.dma_start(out=cs_red[:], in_=cs_dram[:])

            # =============== EMA + laplace ===============
            csr = small.tile([128, K // 128], F32, tag="csr")
            nc.sync.dma_start(out=csr[:], in_=cs_red[0, :].rearrange("(j p) -> p j", p=128))
            cs_in = small.tile([128, K // 128], F32, tag="cs_in")
            nc.sync.dma_start(out=cs_in[:], in_=cs_d[0, :].rearrange("(j p) -> p j", p=128))
            ncs = small.tile([128, K // 128], F32, tag="ncs")
            nc.vector.tensor_scalar_mul(ncs[:], csr[:], 1.0 - DECAY)
            ncs2 = small.tile([128, K // 128], F32, tag="ncs2")
            nc.vector.scalar_tensor_tensor(ncs2[:], cs_in[:], DECAY, ncs[:],
                                           op0=OP.mult, op1=OP.add)
            nc.sync.dma_start(out=ncs_d[0, :].rearrange("(j p) -> p j", p=128), in_=ncs2[:])
            rsum = small.tile([128, 1], F32, tag="rsum")
            nc.vector.tensor_reduce(rsum[:], ncs2[:], axis=AX.X, op=OP.add)
            dsum = small.tile([128, 1], F32, tag="dsum")
            nc.gpsimd.partition_all_reduce(dsum[:], rsum[:], channels=128,
                                           reduce_op=bass_isa.ReduceOp.add)
            nc.vector.tensor_scalar(dsum[:], dsum[:], float(K) * EPS, None, op0=OP.add)
            nrm = small.tile([128, K // 128], F32, tag="nrm")
            nc.vector.tensor_scalar(nrm[:], ncs2[:], EPS, None, op0=OP.add)
            rec = small.tile([128, K // 128], F32, tag="rec")
            nc.vector.reciprocal(rec[:], nrm[:])
            scl = small.tile([128, K // 128], F32, tag="scl")
            nc.vector.tensor_scalar(scl[:], rec[:], dsum[:, 0:1], None, op0=OP.mult)

            NJC = 4
            ear = ea_d.rearrange("(j p) d -> p j d", p=128)
            navgr = navg_d.rearrange("(j p) d -> p j d", p=128)
            nembr = nemb_d.rearrange("(j p) d -> p j d", p=128)
            for c in range((K // 128) // NJC):
                esTs = stage.tile([128, NJC * 128], F32, tag="esTs")
                nc.sync.dma_start(out=esTs[:],
                                  in_=esT_red[:, c * NJC * 128:(c + 1) * NJC * 128])
                esch = stage.tile([128, NJC, D], F32, tag="esch")
                for jj in range(NJC):
                    ptf = ps.tile([128, 128], F32, tag="pb6")
                    nc.tensor.matmul(ptf[:], esTs[:, jj * 128:(jj + 1) * 128], identf[:],
                                     is_transpose=True)
                    nc.scalar.activation(esch[:, jj, :], ptf[:], AF.Copy)
                avch = stage.tile([128, NJC, D], F32, tag="avch")
                nc.sync.dma_start(out=avch[:], in_=ear[:, c * NJC:(c + 1) * NJC, :])
                nvg0 = stage.tile([128, NJC, D], F32, tag="nvg0")
                nc.vector.tensor_scalar_mul(nvg0[:], esch[:], 1.0 - DECAY)
                nvg = stage.tile([128, NJC, D], F32, tag="nvg")
                nc.vector.scalar_tensor_tensor(nvg[:], avch[:], DECAY, nvg0[:],
                                               op0=OP.mult, op1=OP.add)
                nc.sync.dma_start(out=navgr[:, c * NJC:(c + 1) * NJC, :], in_=nvg[:])
                nmb = stage.tile([128, NJC, D], F32, tag="nmb")
                for jj in range(NJC):
                    nc.vector.tensor_scalar(nmb[:, jj, :], nvg[:, jj, :],
                                            scl[:, c * NJC + jj:c * NJC + jj + 1], None,
                                            op0=OP.mult)
                nc.sync.dma_start(out=nembr[:, c * NJC:(c + 1) * NJC, :], in_=nmb[:])

    nc.compile()
    return nc


def _get_nc():
    if "nc" not in _CACHE:
        _CACHE["nc"] = _build()
    return _CACHE["nc"]


def _make_in_maps(inp):
    x = np.ascontiguousarray(np.asarray(inp["x"], dtype=np.float32))
    embed = np.ascontiguousarray(np.asarray(inp["embed"], dtype=np.float32))
    cluster_size = np.ascontiguousarray(np.asarray(inp["cluster_size"], dtype=np.float32))
    embed_avg = np.ascontiguousarray(np.asarray(inp["embed_avg"], dtype=np.float32))
    xf = x.reshape(-1, D)
    e2d = np.ascontiguousarray(embed.reshape(K, D))
    ea2d = np.ascontiguousarray(embed_avg.reshape(K, D))
    cs2d = np.ascontiguousarray(cluster_size.reshape(1, K))
    return [{"x": np.ascontiguousarray(xf[c * T:(c + 1) * T]), "embed": e2d,
             "cluster_size": cs2d, "embed_avg": ea2d} for c in range(NCORES)]


def kernel(x, embed, cluster_size, embed_avg):
    x = np.ascontiguousarray(np.asarray(x, dtype=np.float32))
    embed = np.ascontiguousarray(np.asarray(embed, dtype=np.float32))
    cluster_size = np.ascontiguousarray(np.asarray(cluster_size, dtype=np.float32))
    embed_avg = np.ascontiguousarray(np.asarray(embed_avg, dtype=np.float32))

    B, N, Dd = x.shape
    xf = x.reshape(B * N, Dd)
    e2d = np.ascontiguousarray(embed.reshape(K, D))
    ea2d = np.ascontiguousarray(embed_avg.reshape(K, D))
    cs2d = np.ascontiguousarray(cluster_size.reshape(1, K))

    nc = _get_nc()
    in_maps = []
    for c in range(NCORES):
        in_maps.append({
            "x": np.ascontiguousarray(xf[c * T:(c + 1) * T]),
            "embed": e2d,
            "cluster_size": cs2d,
            "embed_avg": ea2d,
        })
    res = run_bass_kernel_spmd(nc, in_maps, core_ids=list(range(NCORES)))
    outs = res.results

    TT = B * N
    dist = np.empty((TT, K), np.float32)
    indices = np.empty((TT,), np.int32)
    quant = np.empty((TT, D), np.float32)
    for c in range(NCORES):
        r = outs[c]
        dist[c * T:(c + 1) * T] = r["dist"]
        indices[c * T:(c + 1) * T] = r["indices"].reshape(-1)
        quant[c * T:(c + 1) * T] = r["quantized"]
    r0 = outs[0]
    new_embed = r0["new_embed"].reshape(1, K, D)
    new_cs = r0["new_cluster_size"].reshape(1, K)
    new_avg = r0["new_embed_avg"].reshape(1, K, D)

    quantized = quant[:, None, :]
    return quantized, indices, dist, new_embed, new_cs, new_avg


# revision 20
# speedup vs baseline: 47427.0766x; 47427.0766x over previous
"""Trainium2 Bass kernel for EuclideanCodebook (VQ) forward + EMA update.

kernel(**inputs) takes FULL inputs
  x (8, 4096, 128) f32, embed (1, 8192, 128) f32,
  cluster_size (1, 8192) f32, embed_avg (1, 8192, 128) f32
returns the full tuple
  (quantized (32768, 1, 128), indices (32768,) i32, dist (32768, 8192) f32,
   new_embed (1, 8192, 128), new_cluster_size (1, 8192), new_embed_avg (1, 8192, 128))

Sharding: data-parallel over tokens (4096/core x 8 cores), codebook replicated,
per-core partial segment sums all-reduced on-device, EMA computed on-device.

Math per core:
  PSUM p = 2x.e - ||e||^2 via fp16 hi/lo split matmuls (fp32-grade accuracy)
  dist = x2 - p on ScalarE during PSUM->SBUF copy (per-partition bias)
  argmin: per-512-block mins on DVE -> winning 1024-block -> DMA-gather the
  block from the dist output in DRAM -> exact local min + position (max_index)
  segment sums: one-hot (GpSimd) x PE matmuls accumulating es^T / counts
"""
import sys
sys.path.insert(0, "/opt/trn_rl_repo")
import numpy as np

from concourse import bass, bacc, tile, mybir, bass_isa
from concourse.bass_utils import run_bass_kernel_spmd

F32 = mybir.dt.float32
F16 = mybir.dt.float16
I16 = mybir.dt.int16
I32 = mybir.dt.int32
U32 = mybir.dt.uint32
AF = mybir.ActivationFunctionType
OP = mybir.AluOpType
AX = mybir.AxisListType

NCORES = 8
T = 4096
K = 8192
D = 128
NT = T // 128      # 32 token tiles
NKB = K // 512     # 16 k-blocks
DECAY = 0.1
EPS = 1e-5

_CACHE = {}


def _build(num_devices=NCORES, use_collectives=True, do_rescue=True, do_phase2=True, do_distdma=True, do_blockmin=True):
    nc = bacc.Bacc("TRN2", target_bir_lowering=False, debug=False,
                   enable_asserts=False, num_devices=num_devices)
    x_d = nc.dram_tensor("x", [T, D], F32, kind="ExternalInput").ap()
    e_d = nc.dram_tensor("embed", [K, D], F32, kind="ExternalInput").ap()
    cs_d = nc.dram_tensor("cluster_size", [1, K], F32, kind="ExternalInput").ap()
    ea_d = nc.dram_tensor("embed_avg", [K, D], F32, kind="ExternalInput").ap()

    dist_d = nc.dram_tensor("dist", [T, K], F32, kind="ExternalOutput").ap()
    ind_d = nc.dram_tensor("indices", [NT, 128], I32, kind="ExternalOutput").ap()
    qt_d = nc.dram_tensor("quantized", [T, D], F32, kind="ExternalOutput").ap()
    nemb_d = nc.dram_tensor("new_embed", [K, D], F32, kind="ExternalOutput").ap()
    ncs_d = nc.dram_tensor("new_cluster_size", [1, K], F32, kind="ExternalOutput").ap()
    navg_d = nc.dram_tensor("new_embed_avg", [K, D], F32, kind="ExternalOutput").ap()

    with tile.TileContext(nc) as tc:
        with (
            tc.tile_pool(name="cst", bufs=1) as cst,
            tc.tile_pool(name="stage", bufs=2) as stage,
            tc.tile_pool(name="dtile", bufs=5) as dtile,
            tc.tile_pool(name="small", bufs=3) as small,
            tc.tile_pool(name="ps", bufs=1, space="PSUM") as ps,
            tc.tile_pool(name="dram", bufs=1, space="DRAM") as dp,
        ):
            # =============== constants ===============
            ones16 = cst.tile([128, 128], F16, tag="ones16")
            ident = cst.tile([128, 128], F16, tag="ident")
            nc.vector.memset(ones16[:], 1.0)
            nc.gpsimd.affine_select(ident[:], ones16[:], pattern=[[1, 128]],
                                    compare_op=OP.is_equal, fill=0.0,
                                    base=0, channel_multiplier=-1)
            identf = cst.tile([128, 128], F32, tag="identf")
            nc.vector.tensor_copy(identf[:], ident[:])
            ones2 = cst.tile([2, 128], F16, tag="ones2")
            nc.vector.memset(ones2[:], 1.0)
            onescol = cst.tile([128, 1], F16, tag="onescol")
            nc.vector.memset(onescol[:], 1.0)
            qcol = cst.tile([128, 1], F32, tag="qcol")
            nc.vector.memset(qcol[:], 0.25)

            # =============== x: load chunked, split fp16 hi/lo, transpose, x2 ===============
            xhi = cst.tile([128, NT, D], F16, tag="xhi")
            xlo1 = cst.tile([128, NT, D], F16, tag="xlo1")
            xTh = [cst.tile([128, 128], F16, tag="xTh%d" % j, name="xTh%d" % j)
                   for j in range(NT)]
            xTl = [cst.tile([128, 128], F16, tag="xTl%d" % j, name="xTl%d" % j)
                   for j in range(NT)]
            x2s = [cst.tile([128, 1], F32, tag="x2s%d" % j, name="x2s%d" % j)
                   for j in range(NT)]
            NXC = 4
            xr = x_d.rearrange("(j p) d -> p j d", p=128)
            for c in range(NT // NXC):
                x_st = stage.tile([128, NXC, D], F32, tag="x_st")
                nc.sync.dma_start(out=x_st[:], in_=xr[:, c * NXC:(c + 1) * NXC, :])
                jsl = slice(c * NXC, (c + 1) * NXC)
                nc.vector.tensor_copy(xhi[:, jsl, :], x_st[:])
                xlo = stage.tile([128, NXC, D], F16, tag="xlo_st")
                nc.vector.tensor_tensor(xlo[:], x_st[:], xhi[:, jsl, :],
                                        op=OP.subtract)
                nc.vector.tensor_scalar(xlo1[:, jsl, :], xlo[:], 0.125, None,
                                        op0=OP.add)
                for q in range(NXC):
                    j = c * NXC + q
                    scr = stage.tile([128, D], F32, tag="sq_scr")
                    nc.scalar.activation(scr[:], x_st[:, q, :], AF.Square,
                                         accum_out=x2s[j][:])
                    pt = ps.tile([128, 128], F16, tag="pb4")
                    nc.tensor.transpose(pt[:], xhi[:, j, :], ident[:])
                    nc.scalar.activation(xTh[j][:], pt[:], AF.Copy)
                    pt2 = ps.tile([128, 128], F16, tag="pb5")
                    nc.tensor.transpose(pt2[:], xlo[:, q, :], ident[:])
                    nc.scalar.activation(xTl[j][:], pt2[:], AF.Copy)

            # =============== E = 2*embed: load chunked, split, transpose ===============
            ETh = [cst.tile([128, 512], F16, tag="ETh%d" % b, name="ETh%d" % b)
                   for b in range(NKB)]
            ETl = [cst.tile([128, 512], F16, tag="ETl%d" % b, name="ETl%d" % b)
                   for b in range(NKB)]
            NEC = 4
            er = e_d.rearrange("(j p) d -> p j d", p=128)
            for c in range((K // 128) // NEC):
                e_st = stage.tile([128, NEC, D], F32, tag="e_st")
                nc.sync.dma_start(out=e_st[:], in_=er[:, c * NEC:(c + 1) * NEC, :])
                ehi = stage.tile([128, NEC, D], F16, tag="ehi")
                elo = stage.tile([128, NEC, D], F16, tag="elo")
                nc.vector.tensor_scalar_mul(ehi[:], e_st[:], 2.0)
                nc.vector.scalar_tensor_tensor(elo[:], e_st[:], 2.0, ehi[:],
                                               op0=OP.mult, op1=OP.subtract)
                for q in range(NEC):
                    pt = ps.tile([128, 128], F16, tag="pb4")
                    nc.tensor.transpose(pt[:], ehi[:, q, :], ident[:])
                    nc.scalar.activation(ETh[c][:, q * 128:(q + 1) * 128], pt[:], AF.Copy)
                    pt2 = ps.tile([128, 128], F16, tag="pb5")
                    nc.tensor.transpose(pt2[:], elo[:, q, :], ident[:])
                    nc.scalar.activation(ETl[c][:, q * 128:(q + 1) * 128], pt2[:], AF.Copy)

            # =============== -||e||^2 as fp16 (hi, lo) rows ===============
            ne2p = [cst.tile([2, 512], F16, tag="ne2p%d" % b, name="ne2p%d" % b)
                    for b in range(NKB)]
            for b in range(NKB):
                es_ = stage.tile([128, 512], F32, tag="es_")
                nc.vector.tensor_tensor(es_[:], ETh[b][:], ETl[b][:], op=OP.add)
                esq = stage.tile([128, 512], F32, tag="esq")
                nc.vector.tensor_tensor(esq[:], es_[:], es_[:], op=OP.mult)
                pe2 = ps.tile([1, 512], F32, tag="pb6")
                nc.tensor.matmul(pe2[:], qcol[:], esq[:], start=True, stop=True)
                ne2f = stage.tile([1, 512], F32, tag="ne2f")
                nc.scalar.activation(ne2f[:], pe2[:], AF.Copy, scale=-1.0)
                nc.vector.tensor_copy(ne2p[b][0:1, :], ne2f[:])
                ne2lo = stage.tile([1, 512], F16, tag="ne2lo")
                nc.vector.tensor_tensor(ne2lo[:], ne2f[:], ne2p[b][0:1, :],
                                        op=OP.subtract)
                nc.sync.dma_start(out=ne2p[b][1:2, :], in_=ne2lo[:])

            # =============== dist loop ===============
            kf32 = cst.tile([128, NT], F32, tag="kf32")
            kf32s = [cst.tile([128, 1], F32, tag="kfs%d" % j, name="kfs%d" % j)
                     for j in range(NT)]
            b10s = [cst.tile([128, 1], F32, tag="b10s%d" % j, name="b10s%d" % j)
                    for j in range(NT)]
            bmin = cst.tile([128, NT, 8], F32, tag="bmin")
            idxg_dram = dp.tile([NT, 128], I16, tag="idxg_dram")

            for j in range(NT):
                tsl = slice(j * 128, (j + 1) * 128)
                bm512 = small.tile([128, 16], F32, tag="bm512")
                for b in range(NKB):
                    sl = slice(b * 512, (b + 1) * 512)
                    pm = ps.tile([128, 512], F32, tag="pb%d" % (b % 4))
                    nc.tensor.matmul(pm[:], xTh[j][:], ETh[b][:], start=True, stop=False)
                    nc.tensor.matmul(pm[:], xTh[j][:], ETl[b][:], start=False, stop=False)
                    nc.tensor.matmul(pm[:], xTl[j][:], ETh[b][:], start=False, stop=False)
                    nc.tensor.matmul(pm[:], ones2[:], ne2p[b][:], start=False, stop=True)
                    dt = dtile.tile([128, 512], F32, tag="dt")
                    nc.scalar.activation(dt[:], pm[:], AF.Identity,
                                         bias=x2s[j][:], scale=-1.0)
                    if do_distdma:
                        nc.sync.dma_start(out=dist_d[tsl, sl], in_=dt[:])
                    if do_blockmin:
                        nc.vector.tensor_reduce(bm512[:, b:b + 1], dt[:], axis=AX.X, op=OP.min)
                if not do_blockmin:
                    nc.vector.memset(bm512[:], 0.0)
                nc.vector.tensor_tensor(bmin[:, j, :], bm512[:, 0::2], bm512[:, 1::2],
                                        op=OP.min)
                gmin = small.tile([128, 1], F32, tag="gmin")
                nc.vector.tensor_reduce(gmin[:], bmin[:, j, :], axis=AX.X, op=OP.min)
                b10 = small.tile([128, 8], U32, tag="b10")
                nc.vector.max_index(b10[:], gmin[:].broadcast_to([128, 8]), bmin[:, j, :])
                nc.vector.tensor_copy(b10s[j][:], b10[:, 0:1])
                # gather idx = t*8 + b10  (int16; max 32767)
                piota = small.tile([128, 1], F32, tag="piota")
                nc.gpsimd.iota(piota[:], pattern=[[1, 1]], base=j * 128,
                               channel_multiplier=1,
                               allow_small_or_imprecise_dtypes=True)
                gidxf = small.tile([128, 1], F32, tag="gidxf")
                nc.vector.tensor_scalar(gidxf[:], piota[:], 8.0, None, op0=OP.mult)
                nc.vector.tensor_tensor(gidxf[:], gidxf[:], b10s[j][:], op=OP.add)
                gidx16 = small.tile([128, 1], I16, tag="gidx16")
                nc.vector.tensor_copy(gidx16[:], gidxf[:])
                nc.sync.dma_start(out=idxg_dram[j:j + 1, :].rearrange("a p -> p a"),
                                  in_=gidx16[:])

            # =============== rescue: exact position within winning 1024-block ===============
            dist_blocks = dist_d.rearrange("t (b q) -> (t b) q", q=1024)
            for j in (range(NT) if do_rescue else []):
                idxs = small.tile([128, 8], I16, tag="r_idxs")
                for rr in range(8):
                    nc.sync.dma_start(
                        out=idxs[16 * rr:16 * (rr + 1), :],
                        in_=idxg_dram[j:j + 1, :].rearrange("a (s p) -> p (a s)", p=16))
                gbuf = stage.tile([128, 1, 1024], F32, tag="gbuf", bufs=1)
                nc.gpsimd.dma_gather(gbuf[:], dist_blocks, idxs[:], num_idxs=128,
                                     num_idxs_reg=128, elem_size=1024)
                lmin = small.tile([128, 1], F32, tag="lmin")
                nc.vector.tensor_reduce(lmin[:], gbuf[:, 0, :], axis=AX.X, op=OP.min)
                lpos = small.tile([128, 8], U32, tag="lpos")
                nc.vector.max_index(lpos[:], lmin[:].broadcast_to([128, 8]), gbuf[:, 0, :])
                lposf = small.tile([128, 1], F32, tag="lposf")
                nc.vector.tensor_copy(lposf[:], lpos[:, 0:1])
                bb = small.tile([128, 1], F32, tag="bb")
                nc.vector.tensor_scalar(bb[:], b10s[j][:], 1024.0, None,
                                        op0=OP.mult)
                nc.vector.tensor_tensor(kf32s[j][:], bb[:], lposf[:], op=OP.add)
                nc.vector.tensor_copy(kf32[:, j:j + 1], kf32s[j][:])

            if not do_rescue:
                nc.vector.tensor_copy(kf32[:], b10all[:])
            # indices out (int32), token t = 128*j + p  ->  ind_d[j, p]
            ki32 = cst.tile([128, NT], I32, tag="ki32")
            nc.vector.tensor_copy(ki32[:], kf32[:])
            nc.sync.dma_start(out=ind_d.rearrange("j p -> p j"), in_=ki32[:])

            # k as wrapped int16 idx list for the embed gather
            k16 = small.tile([128, NT], I16, tag="k16")
            nc.vector.tensor_copy(k16[:], kf32[:])
            k16_dram = dp.tile([NT, 128], I16, tag="k16_dram")
            nc.sync.dma_start(out=k16_dram[:].rearrange("j p -> p j"), in_=k16[:])
            kidx = cst.tile([128, T // 16], I16, tag="kidx")
            for rr in range(8):
                nc.sync.dma_start(
                    out=kidx[16 * rr:16 * (rr + 1), :],
                    in_=k16_dram[:].rearrange("j (s p) -> p (j s)", p=16))

            # =============== quantized = embed[k] ===============
            NQC = 8   # 1024 idxs per gather
            for c in range(T // (128 * NQC)):
                qbuf = stage.tile([128, NQC, D], F32, tag="qbuf", bufs=1)
                nc.gpsimd.dma_gather(qbuf[:], e_d[:],
                                     kidx[:, c * (128 * NQC) // 16:(c + 1) * (128 * NQC) // 16],
                                     num_idxs=128 * NQC, num_idxs_reg=128 * NQC,
                                     elem_size=D)
                nc.sync.dma_start(
                    out=qt_d.rearrange("(j p) d -> p j d", p=128)[:, c * NQC:(c + 1) * NQC, :],
                    in_=qbuf[:])

            # =============== phase 2: segment sums (es^T, counts) ===============
            esT_dram = dp.tile([128, K], F32, tag="esT_dram")
            cs_dram = dp.tile([1, K], F32, tag="cs_dram")
            if not do_phase2:
                zz512 = stage.tile([128, 512], F32, tag="esd")
                nc.vector.memset(zz512[:], 0.0)
                for b in range(NKB):
                    nc.sync.dma_start(out=esT_dram[:, b * 512:(b + 1) * 512], in_=zz512[:])
                for b in range(0, NKB):
                    nc.sync.dma_start(out=cs_dram[:, b * 512:(b + 1) * 512], in_=zz512[0:1, :])
            for g in (range(4) if do_phase2 else []):
                iotg = stage.tile([128, 2048], F32, tag="iotg", bufs=1)
                nc.gpsimd.iota(iotg[:], pattern=[[1, 2048]], base=g * 2048,
                               channel_multiplier=0,
                               allow_small_or_imprecise_dtypes=True)
                esps = [ps.tile([128, 512], F32, tag="pb%d" % q, name="esps%d" % q) for q in range(4)]
                lops = [ps.tile([128, 512], F32, tag="pb%d" % (4 + q), name="lops%d" % q) for q in range(4)]
                for j in range(NT):
                    oh = stage.tile([128, 2048], F16, tag="oh", bufs=3)
                    eng = nc.gpsimd if (j % 2 == 0) else nc.vector
                    eng.tensor_scalar(oh[:], iotg[:], kf32s[j][:], None,
                                      op0=OP.is_equal)
                    st = (j == 0)
                    sp = (j == NT - 1)
                    for q in range(4):
                        osl = slice(q * 512, (q + 1) * 512)
                        nc.tensor.matmul(esps[q][:], xhi[:, j, :], oh[:, osl],
                                         start=st, stop=sp)
                        nc.tensor.matmul(lops[q][:], xlo1[:, j, :], oh[:, osl],
                                         start=st, stop=sp)
                for q in range(4):
                    b = g * 4 + q
                    sl = slice(b * 512, (b + 1) * 512)
                    # counts: lo row0 = es_lo[0,:] + 0.125*cs ; |8*es_lo| < 0.5
                    csf = stage.tile([1, 512], F32, tag="csf")
                    nc.vector.tensor_scalar(csf[:], lops[q][0:1, :], 8.0, None,
                                            op0=OP.mult)
                    csi = stage.tile([1, 512], I32, tag="csi")
                    nc.vector.tensor_copy(csi[:], csf[:])
                    csd = stage.tile([1, 512], F32, tag="csd")
                    nc.vector.tensor_copy(csd[:], csi[:])
                    nc.sync.dma_start(out=cs_dram[:, sl], in_=csd[:])
                    # store es_hi + es_lo + 0.125*cs; the 0.125*cs correction is
                    # linear, so it is subtracted after the all-reduce (in EMA,
                    # where cs is in per-partition layout).
                    esd = stage.tile([128, 512], F32, tag="esd")
                    nc.scalar.activation(esd[:], esps[q][:], AF.Copy)
                    nc.vector.tensor_tensor(esd[:], esd[:], lops[q][:], op=OP.add)
                    nc.sync.dma_start(out=esT_dram[:, sl], in_=esd[:])

            # =============== all-reduce ===============
            esT_red = dp.tile([128, K], F32, tag="esT_red")
            cs_red = dp.tile([1, K], F32, tag="cs_red")
            if use_collectives:
                nc.gpsimd.collective_compute("AllReduce", OP.add,
                                             replica_groups=[list(range(NCORES))],
                                             ins=[esT_dram[:].opt()], outs=[esT_red[:].opt()])
                nc.gpsimd.collective_compute("AllReduce", OP.add,
                                             replica_groups=[list(range(NCORES))],
                                             ins=[cs_dram[:].opt()], outs=[cs_red[:].opt()])
            else:
                nc.sync.dma_start(out=esT_red[:], in_=esT_dram[:])
                nc.sync.dma_start(out=cs_red[:], in_=cs_dram[:])

            # =============== EMA + laplace ===============
            csr = small.tile([128, K // 128], F32, tag="csr")
            nc.sync.dma_start(out=csr[:], in_=cs_red[0, :].rearrange("(j p) -> p j", p=128))
            cs8 = small.tile([128, K // 128], F32, tag="cs8")
            nc.vector.tensor_scalar(cs8[:], csr[:], 0.125, None, op0=OP.mult)
            cs_in = small.tile([128, K // 128], F32, tag="cs_in")
            nc.sync.dma_start(out=cs_in[:], in_=cs_d[0, :].rearrange("(j p) -> p j", p=128))
            ncs = small.tile([128, K // 128], F32, tag="ncs")
            nc.vector.tensor_scalar_mul(ncs[:], csr[:], 1.0 - DECAY)
            ncs2 = small.tile([128, K // 128], F32, tag="ncs2")
            nc.vector.scalar_tensor_tensor(ncs2[:], cs_in[:], DECAY, ncs[:],
                                           op0=OP.mult, op1=OP.add)
            nc.sync.dma_start(out=ncs_d[0, :].rearrange("(j p) -> p j", p=128), in_=ncs2[:])
            rsum = small.tile([128, 1], F32, tag="rsum")
            nc.vector.tensor_reduce(rsum[:], ncs2[:], axis=AX.X, op=OP.add)
            dsum = small.tile([128, 1], F32, tag="dsum")
            nc.gpsimd.partition_all_reduce(dsum[:], rsum[:], channels=128,
                                           reduce_op=bass_isa.ReduceOp.add)
            nc.vector.tensor_scalar(dsum[:], dsum[:], float(K) * EPS, None, op0=OP.add)
            nrm = small.tile([128, K // 128], F32, tag="nrm")
            nc.vector.tensor_scalar(nrm[:], ncs2[:], EPS, None, op0=OP.add)
            rec = small.tile([128, K // 128], F32, tag="rec")
            nc.vector.reciprocal(rec[:], nrm[:])
            scl = small.tile([128, K // 128], F32, tag="scl")
            nc.vector.tensor_scalar(scl[:], rec[:], dsum[:, 0:1], None, op0=OP.mult)

            NJC = 4
            ear = ea_d.rearrange("(j p) d -> p j d", p=128)
            navgr = navg_d.rearrange("(j p) d -> p j d", p=128)
            nembr = nemb_d.rearrange("(j p) d -> p j d", p=128)
            for c in range((K // 128) // NJC):
                esTs = stage.tile([128, NJC * 128], F32, tag="esTs")
                nc.sync.dma_start(out=esTs[:],
                                  in_=esT_red[:, c * NJC * 128:(c + 1) * NJC * 128])
                esch = stage.tile([128, NJC, D], F32, tag="esch")
                for jj in range(NJC):
                    ptf = ps.tile([128, 128], F32, tag="pb6")
                    nc.tensor.matmul(ptf[:], esTs[:, jj * 128:(jj + 1) * 128], identf[:],
                                     is_transpose=True)
                    nc.scalar.activation(esch[:, jj, :], ptf[:], AF.Copy)
                    nc.vector.tensor_scalar(esch[:, jj, :], esch[:, jj, :],
                                            cs8[:, c * NJC + jj:c * NJC + jj + 1],
                                            None, op0=OP.subtract)
                avch = stage.tile([128, NJC, D], F32, tag="avch")
                nc.sync.dma_start(out=avch[:], in_=ear[:, c * NJC:(c + 1) * NJC, :])
                nvg0 = stage.tile([128, NJC, D], F32, tag="nvg0")
                nc.vector.tensor_scalar_mul(nvg0[:], esch[:], 1.0 - DECAY)
                nvg = stage.tile([128, NJC, D], F32, tag="nvg")
                nc.vector.scalar_tensor_tensor(nvg[:], avch[:], DECAY, nvg0[:],
                                               op0=OP.mult, op1=OP.add)
                nc.sync.dma_start(out=navgr[:, c * NJC:(c + 1) * NJC, :], in_=nvg[:])
                nmb = stage.tile([128, NJC, D], F32, tag="nmb")
                for jj in range(NJC):
                    nc.vector.tensor_scalar(nmb[:, jj, :], nvg[:, jj, :],
                                            scl[:, c * NJC + jj:c * NJC + jj + 1], None,
                                            op0=OP.mult)
                nc.sync.dma_start(out=nembr[:, c * NJC:(c + 1) * NJC, :], in_=nmb[:])

    nc.compile()
    return nc


def _get_nc():
    if "nc" not in _CACHE:
        _CACHE["nc"] = _build()
    return _CACHE["nc"]


def _make_in_maps(inp):
    x = np.ascontiguousarray(np.asarray(inp["x"], dtype=np.float32))
    embed = np.ascontiguousarray(np.asarray(inp["embed"], dtype=np.float32))
    cluster_size = np.ascontiguousarray(np.asarray(inp["cluster_size"], dtype=np.float32))
    embed_avg = np.ascontiguousarray(np.asarray(inp["embed_avg"], dtype=np.float32))
    xf = x.reshape(-1, D)
    e2d = np.ascontiguousarray(embed.reshape(K, D))
    ea2d = np.ascontiguousarray(embed_avg.reshape(K, D))
    cs2d = np.ascontiguousarray(cluster_size.reshape(1, K))
    return [{"x": np.ascontiguousarray(xf[c * T:(c + 1) * T]), "embed": e2d,
             "cluster_size": cs2d, "embed_avg": ea2d} for c in range(NCORES)]


def kernel(x, embed, cluster_size, embed_avg):
    x = np.ascontiguousarray(np.asarray(x, dtype=np.float32))
    embed = np.ascontiguousarray(np.asarray(embed, dtype=np.float32))
    cluster_size = np.ascontiguousarray(np.asarray(cluster_size, dtype=np.float32))
    embed_avg = np.ascontiguousarray(np.asarray(embed_avg, dtype=np.float32))

    B, N, Dd = x.shape
    xf = x.reshape(B * N, Dd)
    e2d = np.ascontiguousarray(embed.reshape(K, D))
    ea2d = np.ascontiguousarray(embed_avg.reshape(K, D))
    cs2d = np.ascontiguousarray(cluster_size.reshape(1, K))

    nc = _get_nc()
    in_maps = []
    for c in range(NCORES):
        in_maps.append({
            "x": np.ascontiguousarray(xf[c * T:(c + 1) * T]),
            "embed": e2d,
            "cluster_size": cs2d,
            "embed_avg": ea2d,
        })
    res = run_bass_kernel_spmd(nc, in_maps, core_ids=list(range(NCORES)))
    outs = res.results

    TT = B * N
    dist = np.empty((TT, K), np.float32)
    indices = np.empty((TT,), np.int32)
    quant = np.empty((TT, D), np.float32)
    for c in range(NCORES):
        r = outs[c]
        dist[c * T:(c + 1) * T] = r["dist"]
        indices[c * T:(c + 1) * T] = r["indices"].reshape(-1)
        quant[c * T:(c + 1) * T] = r["quantized"]
    r0 = outs[0]
    new_embed = r0["new_embed"].reshape(1, K, D)
    new_cs = r0["new_cluster_size"].reshape(1, K)
    new_avg = r0["new_embed_avg"].reshape(1, K, D)

    quantized = quant[:, None, :]
    return quantized, indices, dist, new_embed, new_cs, new_avg
